# revision 3
# baseline (speedup 1.0000x reference)
"""CAMoE GNN layer (GCNConv experts x3, softmax gating) on 8 Trainium2 cores, v2.

Key differences vs v1 (kernel.py):
 - One-hot scatter matrices S are precomputed on host as fp8 (pure index
   preprocessing) and streamed from DRAM, eliminating the per-chunk DVE
   IS_EQ build (327us) that also contended with SWDGE descriptor generation.
 - Target bins are grouped into WAVES whose partial sums accumulate directly
   in PSUM across all 4 source passes (self-loop term enters PSUM via an
   identity-matmul), eliminating the per-group SBUF z accumulation adds.
 - The edge stream is padded only to the cross-core max per (wave, pass, bin)
   cell; 128-chunks may straddle bin boundaries (handled by an extra matmul
   with the same msg chunk), cutting gather indices ~25%.
 - Phase 2 (dinv_t scale, transpose, expert matmuls + gating) is batched per
   wave: one blocked DMA-transpose per wave into an augmented [65, *] lhsT
   (bias via a ones row written over the zero-padded transpose), wave-wide
   DVE combines. Gating (softmax over gate_features @ Wg) is input-only and
   hoisted to a prologue. The DVE combine of wave w is emitted after wave
   w+1's first gather calls so its upstream PE/ACT latency never blocks the
   in-order DVE queue that also feeds the gather message scaling.
"""

import numpy as np

N = 100000
E = 1600000
D = 64
NEXP = 3
GC = 4
TEMP = 101.0
NCORES = 8
P = 128
TILES = 102                 # target bins per core
WAVES = 6
WBINS = TILES // WAVES      # 17 bins per wave
CAP = 123                   # max nodes per bin
SLOTS = TILES * P
NBINS = NCORES * TILES
CHUNK_SRC = 32768
NPASS = (N + CHUNK_SRC - 1) // CHUNK_SRC
SC = 16                     # max 128-edge chunks per dma_gather call
PAD_IDX = 0                 # pad slots gather row 0; dv=0 and no S rows

F32 = np.float32


def _host_prep(edge_index):
    src = edge_index[0].astype(np.int64)
    tgt = edge_index[1].astype(np.int64)
    deg = np.bincount(tgt, minlength=N) + 1
    dinv = (1.0 / np.sqrt(deg.astype(np.float64))).astype(F32)
    indeg = deg - 1

    # --- LPT: nodes -> NBINS bins balanced by in-degree ---
    import heapq
    order = np.argsort(-indeg, kind="stable")
    heap = [(0, b) for b in range(NBINS)]
    heapq.heapify(heap)
    counts = np.zeros(NBINS, np.int64)
    node_bin = np.empty(N, np.int32)
    node_slot = np.empty(N, np.int32)
    deg_l = indeg.tolist()
    for n in order.tolist():
        while True:
            load, b = heapq.heappop(heap)
            if counts[b] < CAP:
                break
        node_bin[n] = b
        node_slot[n] = counts[b]
        counts[b] += 1
        heapq.heappush(heap, (load + deg_l[n], b))

    # --- deal bins to (core, pos) by load rank (snake) ---
    bin_load = np.bincount(node_bin, weights=indeg.astype(np.float64),
                           minlength=NBINS)
    rk = np.argsort(-bin_load, kind="stable")
    bin_core = np.empty(NBINS, np.int64)
    bin_pos = np.empty(NBINS, np.int64)
    for r, b in enumerate(rk.tolist()):
        row, col = divmod(r, NCORES)
        bin_core[b] = col if row % 2 == 0 else NCORES - 1 - col
        bin_pos[b] = row
    bin_wave = bin_pos % WAVES          # rank-dealt -> balanced wave loads
    bin_ws = bin_pos // WAVES           # 0..16, fixed psum slot per bin
    bin_col = bin_wave * WBINS + bin_ws  # column in per-bin arrays

    # --- per-edge attributes ---
    e_bin = node_bin[tgt].astype(np.int64)
    e_core = bin_core[e_bin]
    e_wave = bin_wave[e_bin]
    e_ws = bin_ws[e_bin]
    e_pass = src // CHUNK_SRC

    # --- cross-core-uniform cell sizes: cell = (wave, pass, ws) ---
    cell_key = ((e_core * WAVES + e_wave) * NPASS + e_pass) * WBINS + e_ws
    cell_cnt = np.bincount(cell_key, minlength=NCORES * WAVES * NPASS * WBINS)
    cell_cnt = cell_cnt.reshape(NCORES, WAVES, NPASS, WBINS)
    L_cell = cell_cnt.max(axis=0)                      # [WAVES, NPASS, WBINS]

    # window layout: cells concatenated in ws order, then 128-chunked
    cell_ofs = np.zeros((WAVES, NPASS, WBINS), np.int64)
    C_win = np.zeros((WAVES, NPASS), np.int64)         # chunks per window
    for w in range(WAVES):
        for p in range(NPASS):
            ofs = np.concatenate([[0], np.cumsum(L_cell[w, p])])
            cell_ofs[w, p] = ofs[:-1]
            C_win[w, p] = -(-ofs[-1] // P)
    M_p = C_win.sum(axis=0)                            # chunks per pass
    M_tot = int(C_win.sum())

    # window chunk offsets: stream order is (wave, pass): global chunk index
    win_chunk_ofs = np.zeros((WAVES, NPASS), np.int64)
    ofs = 0
    for w in range(WAVES):
        for p in range(NPASS):
            win_chunk_ofs[w, p] = ofs
            ofs += C_win[w, p]
    # per-pass gidx streams: chunk offset of window within its pass stream
    win_pass_ofs = np.zeros((WAVES, NPASS), np.int64)
    for p in range(NPASS):
        ofs = 0
        for w in range(WAVES):
            win_pass_ofs[w, p] = ofs
            ofs += C_win[w, p]

    # --- mm schedule (uniform across cores) ---
    # per window: list of (chunk_local, ws, mm_id); mm_id global in S stream
    mm_sched = []          # [(wave, pass, chunk_local, ws, mm_id, stop)]
    mm_of_cell = {}        # (w, p, ws) -> list of (chunk_local, mm_id)
    n_mm = 0
    for w in range(WAVES):
        for p in range(NPASS):
            for ws in range(WBINS):
                lo = int(cell_ofs[w, p, ws])
                hi = lo + int(L_cell[w, p, ws])
                if hi == lo:
                    continue
                for c in range(lo // P, (hi - 1) // P + 1):
                    mm_of_cell.setdefault((w, p, ws), []).append((c, n_mm))
                    mm_sched.append([w, p, c, ws, n_mm, False])
                    n_mm += 1
    # stop flags: PSUM accumulation groups are per 2KB bank (8 bins of 64 f32),
    # one group per (wave, bank): stop on the bank's last matmul in the wave
    last_mm = {}
    for ent in mm_sched:
        last_mm[(ent[0], ent[3] // 8)] = ent[4]
    stop_ids = set(last_mm.values())
    for ent in mm_sched:
        ent[5] = ent[4] in stop_ids
    # banks with no edge matmuls at all in a wave must be stopped on their
    # last self-matmul instead; record which (wave, bank) have edge matmuls
    banks_with_mms = {(ent[0], ent[3] // 8) for ent in mm_sched}

    # --- per-core edge placement ---
    # order edges by (core, wave, pass, ws, arbitrary); position within cell
    ekey = ((e_core * WAVES + e_wave) * NPASS + e_pass) * WBINS + e_ws
    eorder = np.argsort(ekey, kind="stable")
    # position within cell for each edge (in eorder): 0,1,2,... per cell
    sk = ekey[eorder]
    starts = np.concatenate([[0], np.nonzero(np.diff(sk))[0] + 1])
    within = np.arange(E, dtype=np.int64)
    within -= np.repeat(starts, np.diff(np.concatenate([starts, [E]])))
    # stream position of each edge within its core's stream
    cw = cell_ofs[e_wave[eorder], e_pass[eorder], e_ws[eorder]]
    wofs = win_chunk_ofs[e_wave[eorder], e_pass[eorder]] * P
    pos = wofs + cw + within                       # [E] in eorder
    ecore_s = e_core[eorder]

    # mm id per edge: via cell's chunk list
    chunk_local = (cw + within) // P
    mm_id = np.empty(E, np.int64)
    # build per-cell lookup arrays: first chunk and first mm of the cell
    cell_first_chunk = np.zeros((WAVES, NPASS, WBINS), np.int64)
    cell_first_mm = np.zeros((WAVES, NPASS, WBINS), np.int64)
    for (w, p, ws), lst in mm_of_cell.items():
        cell_first_chunk[w, p, ws] = lst[0][0]
        cell_first_mm[w, p, ws] = lst[0][1]
    mm_id = (cell_first_mm[e_wave[eorder], e_pass[eorder], e_ws[eorder]]
             + chunk_local - cell_first_chunk[e_wave[eorder], e_pass[eorder],
                                              e_ws[eorder]])

    return dict(
        dinv=dinv, node_bin=node_bin, node_slot=node_slot,
        bin_core=bin_core, bin_col=bin_col,
        L_cell=L_cell, C_win=C_win, M_p=M_p, M_tot=M_tot,
        win_pass_ofs=win_pass_ofs, win_chunk_ofs=win_chunk_ofs,
        mm_sched=mm_sched, n_mm=n_mm, banks_with_mms=banks_with_mms,
        eorder=eorder, ecore_s=ecore_s, pos=pos, mm_id=mm_id,
        src=src, tgt=tgt,
    )


def _core_tensors(prep, k, x_np, gate_features, W, b, Wg, consts):
    import ml_dtypes
    f8 = ml_dtypes.float8_e4m3
    f16 = np.float16

    dinv, node_bin, node_slot = prep["dinv"], prep["node_bin"], prep["node_slot"]
    bin_col = prep["bin_col"]
    M_p, M_tot, n_mm = prep["M_p"], prep["M_tot"], prep["n_mm"]
    sel = prep["ecore_s"] == k
    pos = prep["pos"][sel]
    mm_id = prep["mm_id"][sel]
    eidx = prep["eorder"][sel]
    esrc = prep["src"][eidx]
    etgt = prep["tgt"][eidx]

    # gather index + dinv[src] streams (stream position -> value)
    gidx_all = np.full(M_tot * P, PAD_IDX, np.int64)
    gidx_all[pos] = esrc % CHUNK_SRC
    dv_all = np.zeros(M_tot * P, F32)
    dv_all[pos] = dinv[esrc]
    # split gidx by pass (position ranges are (wave, pass) windows in order)
    C_win = prep["C_win"]
    gidx_p = []
    for p in range(NPASS):
        parts = []
        for w in range(WAVES):
            o = prep["win_chunk_ofs"][w, p] * P
            parts.append(gidx_all[o:o + C_win[w, p] * P])
        ls = np.concatenate(parts) if parts else np.zeros(0, np.int64)
        L = len(ls)
        wrapped = ls.astype(np.int16).reshape(L // 16, 16).T
        gidx_p.append(np.ascontiguousarray(np.tile(wrapped, (8, 1))))
    dsrc = np.ascontiguousarray(dv_all.reshape(M_tot, P).T)

    # S stream: [128, n_mm*128] fp8, S[row, mm*128+slot] = 1
    S_flat = np.zeros(P * n_mm * P, np.uint8)
    row = pos % P
    slot = node_slot[etgt].astype(np.int64)
    S_flat[(row * n_mm + mm_id) * P + slot] = np.asarray(1.0, f8).view(np.uint8)
    S_np = S_flat.reshape(P, n_mm * P).view(f8)

    # per-bin-column arrays
    dinvt = np.zeros((P, TILES), F32)
    gft = np.zeros((GC, SLOTS), F32)
    xself = np.zeros((P, TILES * D), F32)
    mycols = np.nonzero(prep["bin_core"] == k)[0]
    for bg in mycols.tolist():
        j = int(bin_col[bg])
        bnodes = np.nonzero(node_bin == bg)[0]
        sl = node_slot[bnodes]
        dinvt[sl, j] = dinv[bnodes]
        gft[:, j * P + sl] = np.asarray(gate_features)[bnodes].T
        xself[sl, j * D:(j + 1) * D] = x_np[bnodes]

    wcat = np.concatenate([np.asarray(W)[i] for i in range(NEXP)], axis=1)
    wcat_aug = np.concatenate(
        [wcat, np.concatenate([np.asarray(b)[i] for i in range(NEXP)])[None, :]],
        axis=0).astype(f16)

    m = dict(
        S=np.ascontiguousarray(S_np), dsrc=dsrc, dinvt=dinvt,
        gft=np.ascontiguousarray(gft.astype(f16)), xself=xself,
        wcat_aug=np.ascontiguousarray(wcat_aug),
        wg=np.ascontiguousarray(np.asarray(Wg).astype(f16)),
        **consts,
    )
    for p in range(NPASS):
        m[f"gidx{p}"] = gidx_p[p]
    return m


def _build_program(prep):
    import concourse.bass as bass
    import concourse.tile as tile
    from concourse import bacc, mybir

    dt = mybir.dt
    nc = bacc.Bacc("TRN2", target_bir_lowering=False, debug=False,
                   enable_asserts=False, num_devices=NCORES,
                   num_swdge_queues=4)

    C_win, M_p, M_tot, n_mm = prep["C_win"], prep["M_p"], prep["M_tot"], prep["n_mm"]
    mm_sched = prep["mm_sched"]
    win_pass_ofs = prep["win_pass_ofs"]

    xr = [CHUNK_SRC] * (NPASS - 1) + [N - CHUNK_SRC * (NPASS - 1)]
    x_d = [nc.dram_tensor(f"x{p}", [xr[p], D], dt.float32, kind="ExternalInput").ap()
           for p in range(NPASS)]
    gidx_d = [nc.dram_tensor(f"gidx{p}", [P, int(M_p[p]) * 8], dt.int16,
                             kind="ExternalInput").ap() for p in range(NPASS)]
    S_d = nc.dram_tensor("S", [P, n_mm * P], dt.float8e4, kind="ExternalInput").ap()
    dsrc_d = nc.dram_tensor("dsrc", [P, M_tot], dt.float32, kind="ExternalInput").ap()
    dinvt_d = nc.dram_tensor("dinvt", [P, TILES], dt.float32, kind="ExternalInput").ap()
    xself_d = nc.dram_tensor("xself", [P, TILES * D], dt.float32, kind="ExternalInput").ap()
    gft_d = nc.dram_tensor("gft", [GC, SLOTS], dt.float16, kind="ExternalInput").ap()
    wcat_d = nc.dram_tensor("wcat_aug", [D + 1, NEXP * D], dt.float16,
                            kind="ExternalInput").ap()
    wg_d = nc.dram_tensor("wg", [GC, NEXP], dt.float16, kind="ExternalInput").ap()
    ident_d = nc.dram_tensor("ident", [P, P], dt.float8e4, kind="ExternalInput").ap()
    out_d = nc.dram_tensor("out", [SLOTS, D], dt.float32, kind="ExternalOutput").ap()

    # group mms by (wave, pass) window, then slice into gather calls
    win_mms = {}
    for w, p, c, ws, mid, stop in mm_sched:
        win_mms.setdefault((w, p), []).append((c, ws, mid, stop))
    # calls: per window, chunks [c0, c0+SC) etc.
    # call record: (pass, pass_chunk_ofs, global_chunk_ofs, n_chunks,
    #               mm_lo, [(chunk_in_call, ws, stop), ...])
    calls_by_wave = {w: [] for w in range(WAVES)}
    for w in range(WAVES):
        for p in range(NPASS):
            mms = win_mms.get((w, p), [])
            Cw = int(C_win[w, p])
            for c0 in range(0, Cw, SC):
                ck = min(SC, Cw - c0)
                ms = [mm for mm in mms if c0 <= mm[0] < c0 + ck]
                calls_by_wave[w].append((
                    p, int(win_pass_ofs[w, p]) + c0,
                    int(prep["win_chunk_ofs"][w, p]) + c0, ck,
                    ms[0][2] if ms else None,
                    [(mm[0] - c0, mm[1], mm[3], mm[2]) for mm in ms]))

    with tile.TileContext(nc) as tc:
        with tc.tile_pool(name="const", bufs=1) as cpool, \
             tc.tile_pool(name="xsf", bufs=2) as xpool, \
             tc.tile_pool(name="meta", bufs=8) as tpool, \
             tc.tile_pool(name="sbuf", bufs=8) as spool, \
             tc.tile_pool(name="msg", bufs=8) as mpool, \
             tc.tile_pool(name="ms16", bufs=8) as fpool, \
             tc.tile_pool(name="ph2", bufs=2) as kpool, \
             tc.tile_pool(name="pz", bufs=1, space="PSUM") as pz, \
             tc.tile_pool(name="ph", bufs=2, space="PSUM") as ph:

            def load_const(ap_d, shape, tag, dtype=dt.float32):
                t = cpool.tile(shape, dtype, tag=tag)
                nc.sync.dma_start(t[:], ap_d)
                return t

            ident_sb = load_const(ident_d, [P, P], tag="ident", dtype=dt.float8e4)
            wcat_sb = load_const(wcat_d, [D + 1, NEXP * D], tag="wcat", dtype=dt.float16)
            wg_sb = load_const(wg_d, [GC, NEXP], tag="wg", dtype=dt.float16)
            dinvt_sb = load_const(dinvt_d, [P, TILES], tag="dinvt")
            # xselfs = xself * dinvt (the remaining dinv[t] factor is applied
            # with the rest of the bin in phase 2), f16
            xselfs = cpool.tile([P, TILES * D], dt.float16, tag="xselfs")
            for w in range(WAVES):
                xr_t = xpool.tile([P, WBINS * D], dt.float32, tag="xr")
                nc.sync.dma_start(xr_t[:], xself_d[:, w * WBINS * D:(w + 1) * WBINS * D])
                nc.vector.tensor_tensor(
                    out=xselfs[:, w * WBINS * D:(w + 1) * WBINS * D]
                        .rearrange("p (t f) -> p t f", f=D),
                    in0=xr_t[:].rearrange("p (t f) -> p t f", f=D),
                    in1=dinvt_sb[:, w * WBINS:(w + 1) * WBINS].unsqueeze(2)
                        .to_broadcast([P, WBINS, D]),
                    op=mybir.AluOpType.mult,
                )

            # ---- prologue gating: depends only on gate_features ----
            # ge_all = exp(gft.T @ wg / TEMP), gr_all = 1/sum_e ge
            gft_sb = load_const(gft_d, [GC, SLOTS], tag="gft", dtype=dt.float16)
            ge_all = cpool.tile([P, TILES * NEXP], dt.float16, tag="ge_all")
            for w in range(WAVES):
                y_ps = ph.tile([P, WBINS * NEXP], dt.float32, tag="y")
                for ws in range(WBINS):
                    col = w * WBINS + ws
                    nc.tensor.matmul(
                        out=y_ps[:, ws * NEXP:(ws + 1) * NEXP],
                        lhsT=gft_sb[:, col * P:(col + 1) * P],
                        rhs=wg_sb[:], start=(ws == 0), stop=(ws == WBINS - 1))
                nc.scalar.activation(
                    ge_all[:, w * WBINS * NEXP:(w + 1) * WBINS * NEXP], y_ps[:],
                    mybir.ActivationFunctionType.Exp, scale=float(1.0 / TEMP))
            gs_all = cpool.tile([P, TILES], dt.float32, tag="gs_all")
            nc.vector.tensor_reduce(
                out=gs_all[:].unsqueeze(2),
                in_=ge_all[:].rearrange("p (t e) -> p t e", e=NEXP),
                axis=mybir.AxisListType.X, op=mybir.AluOpType.add)
            gr_all = cpool.tile([P, TILES], dt.float32, tag="gr_all")
            nc.vector.reciprocal(gr_all[:], gs_all[:])

            banks_with_mms = prep["banks_with_mms"]
            call_no = 0
            pend = None   # deferred combine of the previous wave

            def combine(w, h, zT):
                # acc = (sum_e ge_e * h_e) * gr for wave w, then DMA out
                acc = kpool.tile([P, WBINS * D], dt.float32, tag="acc")
                tmp = kpool.tile([P, WBINS * D], dt.float32, tag="tmp")
                hv = h[:].rearrange("p (t e f) -> p t e f", e=NEXP, f=D)
                gv = ge_all[:, w * WBINS * NEXP:(w + 1) * WBINS * NEXP] \
                    .rearrange("p (t e) -> p t e", e=NEXP)
                nc.vector.tensor_tensor(
                    out=acc[:].rearrange("p (t f) -> p t f", f=D),
                    in0=hv[:, :, 0, :],
                    in1=gv[:, :, 0].unsqueeze(2).to_broadcast([P, WBINS, D]),
                    op=mybir.AluOpType.mult)
                for i in range(1, NEXP):
                    nc.vector.tensor_tensor(
                        out=tmp[:].rearrange("p (t f) -> p t f", f=D),
                        in0=hv[:, :, i, :],
                        in1=gv[:, :, i].unsqueeze(2).to_broadcast([P, WBINS, D]),
                        op=mybir.AluOpType.mult)
                    nc.vector.tensor_add(acc[:], acc[:], tmp[:])
                nc.vector.tensor_tensor(
                    out=acc[:].rearrange("p (t f) -> p t f", f=D),
                    in0=acc[:].rearrange("p (t f) -> p t f", f=D),
                    in1=gr_all[:, w * WBINS:(w + 1) * WBINS].unsqueeze(2)
                        .to_broadcast([P, WBINS, D]),
                    op=mybir.AluOpType.mult)
                nc.sync.dma_start(
                    out_d[w * WBINS * P:(w + 1) * WBINS * P, :]
                        .rearrange("(t p) f -> p t f", p=P),
                    acc[:].rearrange("p (t f) -> p t f", f=D),
                )

            for w in range(WAVES):
                zw = pz.tile([P, WBINS * D], dt.float32, tag="zw")
                for ws in range(WBINS):
                    bank = ws // 8
                    last_of_bank = (ws == WBINS - 1 or (ws % 8) == 7)
                    nc.tensor.matmul(
                        out=zw[:, ws * D:(ws + 1) * D],
                        lhsT=ident_sb[:],
                        rhs=xselfs[:, (w * WBINS + ws) * D:(w * WBINS + ws + 1) * D],
                        start=(ws % 8 == 0),
                        stop=(last_of_bank and (w, bank) not in banks_with_mms))

                for call_i, (p, pofs, gofs, ck, mm_lo, mms) in enumerate(calls_by_wave[w]):
                    if call_i == 8 and pend is not None:
                        # emit the previous wave's DVE combine here so its
                        # upstream (transpose -> expert matmul -> relu) latency
                        # hides under this wave's first gather calls instead of
                        # stalling the in-order DVE queue
                        combine(*pend)
                        pend = None
                    idx_t = tpool.tile([P, SC * 8], dt.int16, tag="idx")
                    nc.sync.dma_start(idx_t[:, :ck * 8],
                                      gidx_d[p][:, pofs * 8:(pofs + ck) * 8])
                    dv_t = tpool.tile([P, SC], dt.float32, tag="dv")
                    nc.sync.dma_start(dv_t[:, :ck], dsrc_d[:, gofs:gofs + ck])
                    n_call_mm = len(mms)
                    S_t = spool.tile([P, (SC + WBINS) * P], dt.float8e4, tag="S")
                    if n_call_mm:
                        nc.sync.dma_start(
                            S_t[:, :n_call_mm * P],
                            S_d[:, mm_lo * P:(mm_lo + n_call_mm) * P])
                    msg = mpool.tile([P, SC * D], dt.float32, tag="msg")
                    if call_no < 8:
                        # pad slots (idx=-1) are skipped by the gather ucode;
                        # zero the first rotation of msg buffers so stale SBUF
                        # bits can't inject NaN into the 0-weighted matmul rows
                        nc.vector.memset(msg[:], 0.0)
                    nc.gpsimd.dma_gather(
                        out_ap=msg[:, :ck * D].rearrange("p (c f) -> p c f", f=D),
                        in_ap=x_d[p],
                        idxs_ap=idx_t[:, :ck * 8],
                        num_idxs=ck * P,
                        num_idxs_reg=ck * P,
                        elem_size=D,
                        single_packet=False,
                        queue_num=call_no % 4,
                    )
                    call_no += 1
                    ms = fpool.tile([P, SC * D], dt.float16, tag="ms")
                    nc.vector.tensor_tensor(
                        out=ms[:, :ck * D].rearrange("p (c f) -> p c f", f=D),
                        in0=msg[:, :ck * D].rearrange("p (c f) -> p c f", f=D),
                        in1=dv_t[:, :ck].unsqueeze(2).to_broadcast([P, ck, D]),
                        op=mybir.AluOpType.mult,
                    )
                    for (ci, ws, stop, mid) in mms:
                        nc.tensor.matmul(
                            out=zw[:, ws * D:(ws + 1) * D],
                            lhsT=S_t[:, (mid - mm_lo) * P:(mid - mm_lo + 1) * P],
                            rhs=ms[:, ci * D:(ci + 1) * D],
                            start=False, stop=stop)

                # ---- phase 2 for wave w ----
                # zs: one 128-wide block per bin; cols [0:64) = z * dinvt,
                # cols [64:128) zeroed so the transpose leaves partitions
                # 64..127 of zT zero (row 64 then becomes the bias ones row)
                zs = kpool.tile([P, WBINS * P], dt.float16, tag="zs")
                zsv = zs[:].rearrange("p (t f) -> p t f", f=P)
                nc.vector.memset(zsv[:, :, D:P], 0.0)
                nc.vector.tensor_tensor(
                    out=zsv[:, :, 0:D],
                    in0=zw[:].rearrange("p (t f) -> p t f", f=D),
                    in1=dinvt_sb[:, w * WBINS:(w + 1) * WBINS].unsqueeze(2)
                        .to_broadcast([P, WBINS, D]),
                    op=mybir.AluOpType.mult,
                )
                zT = kpool.tile([P, WBINS * P], dt.float16, tag="zT")
                nc.sync.dma_start_transpose(
                    zT[:].rearrange("p (t c) -> p t c", c=P), zs[:])
                nc.vector.memset(zT[D:D + 1, :], 1.0)
                h = kpool.tile([P, WBINS * NEXP * D], dt.float16, tag="h")
                for ws in range(WBINS):
                    h_ps = ph.tile([P, NEXP * D], dt.float32, tag="h")
                    nc.tensor.matmul(
                        out=h_ps[:], lhsT=zT[0:D + 1, ws * P:(ws + 1) * P],
                        rhs=wcat_sb[:], start=True, stop=True)
                    nc.scalar.activation(h[:, ws * NEXP * D:(ws + 1) * NEXP * D],
                                         h_ps[:], mybir.ActivationFunctionType.Relu)
                pend = (w, h, zT)
            combine(*pend)
    nc.finalize()
    return nc


def _consts():
    import ml_dtypes
    return dict(ident=np.eye(P, dtype=np.float32).astype(ml_dtypes.float8_e4m3))


def kernel(x, edge_index, gate_features, W, b, Wg):
    from concourse.bass_utils import run_bass_kernel_spmd

    x = np.ascontiguousarray(np.asarray(x), dtype=F32)
    edge_index = np.asarray(edge_index)
    prep = _host_prep(edge_index)
    consts = _consts()
    in_maps = [_core_tensors(prep, k, x, gate_features, W, b, Wg, consts)
               for k in range(NCORES)]
    for p in range(NPASS):
        xp = np.ascontiguousarray(x[p * CHUNK_SRC:min((p + 1) * CHUNK_SRC, N)])
        for m in in_maps:
            m[f"x{p}"] = xp
    nc = _build_program(prep)
    res = run_bass_kernel_spmd(nc, in_maps, core_ids=list(range(NCORES)))
    global LAST_RESULTS
    LAST_RESULTS = res
    node_bin, node_slot = prep["node_bin"], prep["node_slot"]
    rows = prep["bin_col"][node_bin] * P + node_slot
    per_core = np.stack([res.results[k]["out"] for k in range(NCORES)])
    out = per_core[prep["bin_core"][node_bin], rows]
    return np.ascontiguousarray(out)
